# revision 6
# baseline (speedup 1.0000x reference)
"""VQ codebook top-k kernel for Trainium2 (8 NeuronCores, data-parallel over x rows).

Problem: x (8192,768) fp32, codebook (32768,768) fp32, k=32.
  cos_sim = normalize(x) @ normalize(codebook).T ; top-32 per row; sum gathered rows.

Per core: 1024 x-rows, full codebook.
Algorithm:
  - x normalization skipped (positive per-row scale never changes that row's top-k).
  - codebook rows normalized on-chip, split into bf16 hi/lo (hi=bf16(v), lo=bf16(v-hi)),
    written to DRAM, transpose-loaded via DMA xbar as [d,n] tiles.
  - similarity via 3-product bf16 split matmul (hi*hi + hi*lo + lo*hi) accumulated
    in fp32 PSUM -> ~1e-6 relative accuracy (rank-safe; boundary gaps ~3e-4).
  - top-8 per 512-chunk via DVE max/max_index (covers top-32: verified offline, P(fail)~1e-3).
  - merge: threshold tau = 32nd candidate value (4x max+match_replace rounds), then
    extract selected global indices from enc = 40000 - gidx via 4 more max rounds
    (exact integer fp32 arithmetic; avoids per-partition gather, which HW DGE lacks).
  - gather+sum: 32 indirect DMAs per 128-row batch (one row per partition) + DVE adds.
"""
import os
import numpy as np
from contextlib import ExitStack

import concourse.bass as bass
import concourse.bacc as bacc
import concourse.tile as tile
from concourse import mybir
from concourse.bass_utils import run_bass_kernel_spmd

F32 = mybir.dt.float32
BF16 = mybir.dt.bfloat16
U32 = mybir.dt.uint32

M_CORE = 1024        # x rows per core
N = 32768            # codebook rows
D = 768              # embedding dim
K = 32               # top-k
KT = D // 128        # 6 K-tiles
NCH = N // 512       # 64 chunks
MB = M_CORE // 128   # 8 m-batches
ENC0 = 40000.0       # enc = ENC0 - gidx  (exact in fp32, gidx < 32768)

_CACHE = {}


def _build_kernel(M_CORE=M_CORE, N=N, D=D):
    KT = D // 128
    NCH = N // 512
    MB = M_CORE // 128
    nc = bacc.Bacc("TRN2", target_bir_lowering=False, debug=False)
    x = nc.dram_tensor("x", (M_CORE, D), F32, kind="ExternalInput").ap()
    cb = nc.dram_tensor("cb", (N, D), F32, kind="ExternalInput").ap()
    xhat = nc.dram_tensor("xhat", (M_CORE, D), F32, kind="ExternalOutput").ap()
    # DRAM scratch for normalized bf16 hi/lo codebook (natural layout)
    cbh_d = nc.dram_tensor("cbh_d", (N, D), BF16, kind="Internal").ap()
    cbl_d = nc.dram_tensor("cbl_d", (N, D), BF16, kind="Internal").ap()

    with tile.TileContext(nc) as tc, ExitStack() as ctx:
        pool = ctx.enter_context(tc.tile_pool(name="sbuf", bufs=3))
        cpool = ctx.enter_context(tc.tile_pool(name="cbt", bufs=2))
        pers = ctx.enter_context(tc.tile_pool(name="pers", bufs=1))
        spool = ctx.enter_context(tc.tile_pool(name="sel", bufs=2))
        gpool = ctx.enter_context(tc.tile_pool(name="gath", bufs=4))
        psum = ctx.enter_context(tc.tile_pool(name="psum", bufs=8, space="PSUM"))

        # ---------------- x prep: bf16 split + transpose (no normalization) --------
        xTh = [pers.tile([128, M_CORE], BF16, name=f"xTh{i}") for i in range(KT)]
        xTl = [pers.tile([128, M_CORE], BF16, name=f"xTl{i}") for i in range(KT)]
        for m in range(MB):
            xt = pool.tile([128, D], F32, tag="xt")
            nc.sync.dma_start(xt[:], x[m * 128:(m + 1) * 128, :])
            xh = pool.tile([128, D], BF16, tag="xh")
            xl = pool.tile([128, D], BF16, tag="xl")
            nc.scalar.copy(xh[:], xt[:])
            nc.vector.tensor_sub(xl[:], xt[:], xh[:])
            for kd in range(KT):
                nc.sync.dma_start_transpose(
                    xTh[kd][:, m * 128:(m + 1) * 128], xh[:, kd * 128:(kd + 1) * 128])
                nc.sync.dma_start_transpose(
                    xTl[kd][:, m * 128:(m + 1) * 128], xl[:, kd * 128:(kd + 1) * 128])

        # ---------------- candidate arrays (per m-batch) ---------------------------
        cand_val = [pers.tile([128, NCH * 8], F32, name=f"cv{i}") for i in range(MB)]
        cand_enc = [pers.tile([128, NCH * 8], F32, name=f"ce{i}") for i in range(MB)]

        # ---------------- codebook stream ------------------------------------------
        for c in range(NCH):
            # prep 512 rows: normalize + split, park in DRAM
            for b in range(4):
                r0 = c * 512 + b * 128
                cbb = pool.tile([128, D], F32, tag="cbb")
                nc.sync.dma_start(cbb[:], cb[r0:r0 + 128, :])
                sq = pool.tile([128, D], F32, tag="sq")
                nsq = pool.tile([128, 1], F32, tag="nsq")
                nc.scalar.activation(sq[:], cbb[:], mybir.ActivationFunctionType.Square,
                                     accum_out=nsq[:])
                norm = pool.tile([128, 1], F32, tag="norm")
                nc.scalar.activation(norm[:], nsq[:], mybir.ActivationFunctionType.Sqrt)
                rnorm = pool.tile([128, 1], F32, tag="rnorm")
                nc.vector.reciprocal(rnorm[:], norm[:])
                cbn = pool.tile([128, D], F32, tag="cbn")
                nc.vector.tensor_scalar_mul(cbn[:], cbb[:], rnorm[:])
                cbh = pool.tile([128, D], BF16, tag="cbh")
                nc.scalar.copy(cbh[:], cbn[:])
                cbl = pool.tile([128, D], BF16, tag="cbl")
                nc.vector.tensor_sub(cbl[:], cbn[:], cbh[:])
                nc.scalar.dma_start(cbh_d[r0:r0 + 128, :], cbh[:])
                nc.scalar.dma_start(cbl_d[r0:r0 + 128, :], cbl[:])

            # transpose-load [d, n] tiles for this chunk
            cbTh = cpool.tile([128, KT * 512], BF16, tag="cbTh")
            cbTl = cpool.tile([128, KT * 512], BF16, tag="cbTl")
            for kd in range(KT):
                nc.sync.dma_start_transpose(
                    cbTh[:, kd * 512:(kd + 1) * 512],
                    cbh_d[c * 512:(c + 1) * 512, kd * 128:(kd + 1) * 128])
                nc.sync.dma_start_transpose(
                    cbTl[:, kd * 512:(kd + 1) * 512],
                    cbl_d[c * 512:(c + 1) * 512, kd * 128:(kd + 1) * 128])

            # matmuls + per-chunk top-8
            for m in range(MB):
                ps = psum.tile([128, 512], F32, tag="ps")
                i = 0
                for kd in range(KT):
                    xh_t = xTh[kd][:, m * 128:(m + 1) * 128]
                    xl_t = xTl[kd][:, m * 128:(m + 1) * 128]
                    ch_t = cbTh[:, kd * 512:(kd + 1) * 512]
                    cl_t = cbTl[:, kd * 512:(kd + 1) * 512]
                    for lh, rh in ((xh_t, ch_t), (xh_t, cl_t), (xl_t, ch_t)):
                        nc.tensor.matmul(ps[:], lh, rh, start=(i == 0), stop=(i == KT * 3 - 1))
                        i += 1
                s_sb = pool.tile([128, 512], F32, tag="s_sb")
                nc.scalar.copy(s_sb[:], ps[:])
                cv8 = cand_val[m][:, c * 8:(c + 1) * 8]
                nc.vector.max(cv8, s_sb[:])
                pos8 = pool.tile([128, 8], U32, tag="pos8")
                nc.vector.max_index(pos8[:], cv8, s_sb[:])
                posf = pool.tile([128, 8], F32, tag="posf")
                nc.vector.tensor_copy(posf[:], pos8[:])
                # enc = (ENC0 - c*512) - pos
                nc.vector.tensor_scalar(
                    cand_enc[m][:, c * 8:(c + 1) * 8], posf[:],
                    -1.0, scalar2=float(ENC0 - c * 512),
                    op0=mybir.AluOpType.mult, op1=mybir.AluOpType.add)

        # ---------------- merge + gather + output ---------------------------------
        for m in range(MB):
            # tau = 32nd largest candidate value
            scr = spool.tile([128, NCH * 8], F32, tag="scr")
            nc.vector.tensor_copy(scr[:], cand_val[m][:])
            v8 = None
            for r in range(4):
                v8 = spool.tile([128, 8], F32, tag="v8")
                nc.vector.max(v8[:], scr[:])
                if r < 3:
                    nc.vector.match_replace(scr[:], in_to_replace=v8[:],
                                            in_values=scr[:], imm_value=-1e30)
            tau = v8[:, 7:8]
            # selected mask * enc
            mask = spool.tile([128, NCH * 8], F32, tag="mask")
            nc.vector.tensor_scalar(mask[:], cand_val[m][:], tau,
                                    scalar2=None, op0=mybir.AluOpType.is_ge)
            arr = spool.tile([128, NCH * 8], F32, tag="arr")
            nc.vector.tensor_mul(arr[:], mask[:], cand_enc[m][:])
            # extract 32 selected enc values
            sel_enc = spool.tile([128, K], F32, tag="sel_enc")
            for r in range(4):
                e8 = sel_enc[:, r * 8:(r + 1) * 8]
                nc.vector.max(e8, arr[:])
                if r < 3:
                    nc.vector.match_replace(arr[:], in_to_replace=e8,
                                            in_values=arr[:], imm_value=0.0)
            # decode gidx = ENC0 - enc
            gidxf = spool.tile([128, K], F32, tag="gidxf")
            nc.vector.tensor_scalar(gidxf[:], sel_enc[:], -1.0, scalar2=ENC0,
                                    op0=mybir.AluOpType.mult, op1=mybir.AluOpType.add)
            sel = spool.tile([128, K], U32, tag="sel")
            nc.vector.tensor_copy(sel[:], gidxf[:])

            # gather + sum
            acc = spool.tile([128, D], F32, tag="acc")
            for j in range(K):
                g = gpool.tile([128, D], F32, tag="g")
                nc.gpsimd.indirect_dma_start(
                    out=g[:], out_offset=None, in_=cb[:],
                    in_offset=bass.IndirectOffsetOnAxis(ap=sel[:, j:j + 1], axis=0))
                if j == 0:
                    nc.vector.tensor_copy(acc[:], g[:])
                else:
                    nc.vector.tensor_add(acc[:], acc[:], g[:])
            nc.sync.dma_start(xhat[m * 128:(m + 1) * 128, :], acc[:])

    nc.compile()
    return nc


def kernel(**inputs):
    x = np.ascontiguousarray(np.asarray(inputs["x"], dtype=np.float32))
    cb = np.ascontiguousarray(np.asarray(inputs["codebook"], dtype=np.float32))
    k = int(np.asarray(inputs["k"]))
    assert x.shape == (8192, 768) and cb.shape == (32768, 768) and k == 32

    if "nc" not in _CACHE:
        _CACHE["nc"] = _build_kernel()
    nc = _CACHE["nc"]

    in_maps = [{"x": x[i * M_CORE:(i + 1) * M_CORE], "cb": cb} for i in range(8)]
    res = run_bass_kernel_spmd(nc, in_maps, core_ids=list(range(8)),
                               trace=bool(int(os.environ.get("VQ_TRACE", "0"))))
    _CACHE["last_result"] = res
    out = np.concatenate([res.results[i]["xhat"] for i in range(8)], axis=0)
    return out.astype(np.float32)
